# revision 33
# baseline (speedup 1.0000x reference)
"""Trainium2 Bass kernel for nn_CrossAttention (B=8, Sq=Skv=2048, D=1024, C=768).

Strategy: data-parallel over batch — each of the 8 NeuronCores computes one
batch element's full cross-attention.

The projection chain is reassociated so every big contraction runs against
the NARROW context dim (C=768) instead of D=1024, and the K/V projections
disappear entirely (all exact identities, weights folded on host):

  scores = (x @ M + bqk) @ ctx^T          M   = Wq^T @ Wk   [D, C]
                                          bqk = bq @ Wk     [C]
  (bk drops: its score term is constant over k -> cancels in softmax)
  att    = softmax(scores / sqrt(D))
  final  = (e @ ctx)/sums @ WVO + bo''    WVO = (Wo @ Wv)^T [C, D]
                                          bo''= bo + Wo @ bv

Precision plan (2e-2 gate; measures 1.67e-2 end-to-end):
  - scores matmul: fp8e4m3 DoubleRow (xm8 @ ctx8), 1/sqrt(D) folded into Exp.
  - att @ ctx matmul: fp8 DoubleRow with CENTERED attention weights:
      e_hat = EC + e8/ESC,  e8 = fp8((e - EC) * ESC)
    e clusters tightly around its mean (~1.1, std 0.52), so quantizing the
    centered residual cuts the fp8 noise ~3x vs direct fp8(e). The rank-1
    correction EC * colsum(ctxk8) is folded into the PSUM-evac bias
    (host-computed from the quantized ctxk8, exact, free). The softmax
    denominator is accumulated from the SAME quantized e8 values
    (consistent normalization); sums_true = 2048*EC + (sum e8)/ESC is
    folded into the reciprocal stage.
  - phase-1 projection and the final @WVO stay bf16: fp8 there busts the
    error budget (emulated 2.1e-2 / 2.5e-2 vs the 2e-2 gate).
  - output written bf16 (halves the tail DMA; +3e-4 error), host casts f32.

Per-core structure (772 matmuls, ~169us PE-dense of ~192us total):
  phase 1 (q chunks of 512): xm^T[c,q] = M^T x^T + bqk -> fp8 resident.
    Chunk 0 runs it-OUTER across 6 live psum banks so each arriving
    (M[it], x0[it]) DMA pair immediately feeds 6 matmuls; psum evacs ride
    VECTOR (tensor_scalar add-bias -> fp8), keeping the scalar/ACT stream
    empty so it can never head-of-line-block (see DMA note below).
  phase 2, SOFTWARE-PIPELINED per q block qb:
    scores(qb):  psc fp8-DR -> Exp(bf16, ACT) -> center(fp8, DVE);
                 pair-sums of e8 on gpsimd + DVE add-tree.
    finals(qb-1): outp^T.T @ WVO (bf16) -> STT (*recip + bo'') -> bf16 DMA.
                 Running these between scores(qb) and outp(qb) hides the
                 Exp->center pipeline drain that outp_col(0) (226ns/pair
                 consumption vs 1.36us/pair production) would otherwise
                 catch up with.
    outp(qb):    outp^T[c,q] = ctxk8^T @ e8 (fp8 DR over kt pairs); evac
                 fuses *1/ESC + EC*colsum bias. The ones-matmul +
                 e0-trick transpose + reciprocal interleave between outp
                 columns so the PE never waits on the sums chain.

DMA: dma_start instructions enqueue descriptors into per-engine SW-DGE
queue rings (~87GB/s each) and BLOCK the issuing engine when a ring is
full. Only sync/gpsimd/scalar can issue. Since no compute rides those
three engines before qb0's Exp (~59us), all three carry a full
round-robin share of the input stream in need order (M+x0 first). Output
tiles go out on sync+gpsimd only, keeping scalar's Exp cadence clean.
"""

import numpy as np
import ml_dtypes

import concourse.bass as bass  # noqa: F401
import concourse.mybir as mybir
import concourse.tile as tile
from concourse import bacc
from concourse.bass_utils import run_bass_kernel_spmd

# ---- problem shapes (hardcoded) ----
B, SQ, SKV, D, C = 8, 2048, 2048, 1024, 768
P = 128
DT = D // P          # 8  d-tiles
CT = C // P          # 6  c-tiles
KT = SKV // P        # 16 k-tiles
QB = 512             # q block width
NQB = SQ // QB       # 4 q blocks
SCALE = 1.0 / np.sqrt(np.float32(D))

EC = 1.105           # centering constant for e = exp(score)
ESC = 8.0            # fp8 scale for the centered residual

F32 = mybir.dt.float32
BF16 = mybir.dt.bfloat16
FP8 = mybir.dt.float8e4
AF = mybir.ActivationFunctionType
ALU = mybir.AluOpType
DR = mybir.MatmulPerfMode.DoubleRow

_NC_CACHE = {}


def build():
    if "nc" in _NC_CACHE:
        return _NC_CACHE["nc"]
    nc = bacc.Bacc(trn_type="TRN2", num_swdge_queues=4)

    # ---- DRAM I/O (per-core slices; names = in_map keys) ----
    xT = nc.dram_tensor("xT", [D, SQ], BF16, kind="ExternalInput")
    ctx8T = nc.dram_tensor("ctx8T", [C, SKV], FP8, kind="ExternalInput")
    ctxk8 = nc.dram_tensor("ctxk8", [SKV, C], FP8, kind="ExternalInput")
    Mh = nc.dram_tensor("Mh", [D, C], BF16, kind="ExternalInput")
    wvoh = nc.dram_tensor("wvoh", [C, D], BF16, kind="ExternalInput")
    bqkh = nc.dram_tensor("bqkh", [P, CT], F32, kind="ExternalInput")
    bob = nc.dram_tensor("bob", [P, D], F32, kind="ExternalInput")
    csumb = nc.dram_tensor("csumb", [P, CT], F32, kind="ExternalInput")
    onesmat = nc.dram_tensor("onesmat", [P, P], BF16, kind="ExternalInput")
    e0two = nc.dram_tensor("e0two", [P, 2], BF16, kind="ExternalInput")
    ecbh = nc.dram_tensor("ecbh", [P, 1], F32, kind="ExternalInput")
    out = nc.dram_tensor("out", [SQ, D], BF16, kind="ExternalOutput")

    with tile.TileContext(nc) as tc:
        with tc.tile_pool(name="persist", bufs=1) as persist:
            ctx8_sb = persist.tile([P, CT, SKV], FP8, name="ctx8_sb")
            ctxk_sb = persist.tile([P, KT, C], FP8, name="ctxk_sb")
            xm_tiles = [persist.tile([P, CT, QB], FP8,
                                     name=f"xm_sb{qc}")
                        for qc in range(NQB)]
            m_tiles = [None] + [persist.tile([P, C], BF16,
                                             name=f"m_sb{it}")
                                for it in range(1, DT)]
            m0a = persist.tile([P, 384], BF16, name="m0a")
            m0b = persist.tile([P, 384], BF16, name="m0b")
            xt_tiles = [[persist.tile([P, QB], BF16, name=f"xt{qc}_{it}")
                         for it in range(DT)] for qc in range(NQB)]
            wvo_sb = persist.tile([P, CT, D], BF16, name="wvo_sb")
            bqk_sb = persist.tile([P, CT], F32, name="bqk_sb")
            bo_sb = persist.tile([P, D], F32, name="bo_sb")
            csum_sb = persist.tile([P, CT], F32, name="csum_sb")
            om_sb = persist.tile([P, P], BF16, name="om_sb")
            e0_sb = persist.tile([P, 2], BF16, name="e0_sb")
            ec_sb = persist.tile([P, 1], F32, name="ec_sb")
            sums_sb = persist.tile([P, QB], BF16, name="sums_sb")

            # ---- DMA plan ----
            # phase-1 evacs ride VECTOR, so all three issue-capable
            # engines (sync/gpsimd/scalar) can carry a full round-robin
            # DMA share; ring backpressure never blocks compute (scalar's
            # first compute op is qb0's Exp at ~59us, long after its ring
            # drains). Pieces are issued in need order.
            engs = (nc.sync, nc.gpsimd, nc.scalar)
            ei = 0

            def issue(dst, src):
                nonlocal ei
                engs[ei % 3].dma_start(dst, src)
                ei += 1

            # first pieces ordered by first-use: x00 halves, m0 halves
            issue(xt_tiles[0][0][:, 0:256], xT[0:P, 0:256])
            issue(xt_tiles[0][0][:, 256:512], xT[0:P, 256:512])
            issue(m0a, Mh[0:P, 0:384])
            issue(m0b, Mh[0:P, 384:768])
            issue(bqk_sb, bqkh[:])
            for it in range(DT):
                if it == 0:
                    pass
                else:
                    issue(m_tiles[it], Mh[it * P:(it + 1) * P, :])
                    issue(xt_tiles[0][it], xT[it * P:(it + 1) * P, 0:QB])
            for qc in range(1, NQB):
                for it in range(DT):
                    issue(xt_tiles[qc][it],
                          xT[it * P:(it + 1) * P, qc * QB:(qc + 1) * QB])
            for t in range(CT):
                issue(ctx8_sb[:, t], ctx8T[t * P:(t + 1) * P, :])
            for kt_ in range(KT):
                issue(ctxk_sb[:, kt_], ctxk8[kt_ * P:(kt_ + 1) * P, :])
            for t in range(CT):
                issue(wvo_sb[:, t], wvoh[t * P:(t + 1) * P, :])
            issue(csum_sb, csumb[:])
            issue(bo_sb, bob[:])
            issue(om_sb, onesmat[:])
            issue(e0_sb, e0two[:])
            issue(ec_sb, ecbh[:])

            # ===== phase 1: xm^T[c,q] = M^T @ x^T (+bqk), fp8 resident =====
            with tc.tile_pool(name="ps_xm", bufs=1, space="PSUM") as ps_xm:
                # chunk 0: it-outer across 6 live psum banks so the PE
                # starts as soon as the first (M[it], x[it]) pair lands
                pxms = [ps_xm.tile([P, QB], F32, name=f"pxm{cs}",
                                   tag=f"pxm{cs}") for cs in range(CT)]
                def mslice(it, cs):
                    if it == 0:
                        return (m0a[:, cs * P:(cs + 1) * P] if cs < 3
                                else m0b[:, (cs - 3) * P:(cs - 2) * P])
                    return m_tiles[it][:, cs * P:(cs + 1) * P]
                for it in range(DT):
                    for cs in range(CT):
                        nc.tensor.matmul(
                            pxms[cs], mslice(it, cs), xt_tiles[0][it],
                            start=(it == 0), stop=(it == DT - 1))
                for cs in range(CT):
                    nc.vector.tensor_scalar(
                        xm_tiles[0][:, cs], pxms[cs],
                        bqk_sb[:, cs:cs + 1], 1.0,
                        op0=ALU.add, op1=ALU.mult)
                # chunks 1..3: cs-outer (DMA is ahead; evac WAR long gone)
                for qc in range(1, NQB):
                    for cs in range(CT):
                        pxm = ps_xm.tile([P, QB], F32, name=f"pxm{cs}",
                                         tag=f"pxm{cs}")
                        for it in range(DT):
                            nc.tensor.matmul(
                                pxm, mslice(it, cs), xt_tiles[qc][it],
                                start=(it == 0), stop=(it == DT - 1))
                        nc.vector.tensor_scalar(
                            xm_tiles[qc][:, cs], pxm,
                            bqk_sb[:, cs:cs + 1], 1.0,
                            op0=ALU.add, op1=ALU.mult)

            # ================= phase 2: attention + fold-out ================
            with tc.tile_pool(name="p2_big", bufs=1) as p2_big, \
                 tc.tile_pool(name="p2_acc", bufs=16) as p2_acc, \
                 tc.tile_pool(name="ps_sc", bufs=2, space="PSUM") as ps_sc, \
                 tc.tile_pool(name="ps_po", bufs=2, space="PSUM") as ps_po, \
                 tc.tile_pool(name="ps_fin", bufs=2, space="PSUM") as ps_fin:
                # per-pair expt tiles: outp's kp-call then depends only on
                # its own pair's centering op, not all eight (dependency
                # tracking is per-tile)
                expt_tiles = [p2_big.tile([P, 2, QB], FP8, name=f"expt{kp}")
                              for kp in range(KT // 2)]
                outp_sb = p2_big.tile([P, CT, QB], BF16, name="outp_sb")
                def scores_block(qb):
                    pairs = []
                    for kp in range(KT // 2):
                        psc = ps_sc.tile([P, 2, QB], F32, name="psc",
                                         tag="psc")
                        for j in range(2):
                            kt_ = kp * 2 + j
                            for cs in range(0, CT, 2):
                                nc.tensor.matmul(
                                    psc[:, j],
                                    ctx8_sb[:, cs:cs + 2,
                                            kt_ * P:(kt_ + 1) * P],
                                    xm_tiles[qb][:, cs:cs + 2],
                                    start=(cs == 0), stop=(cs == CT - 2),
                                    perf_mode=DR)
                        tmp = p2_acc.tile([P, 2, QB], BF16, name="tmp",
                                          tag="tmp")
                        nc.scalar.activation(tmp, psc, AF.Exp,
                                             scale=float(SCALE))
                        nc.vector.tensor_scalar(
                            expt_tiles[kp], tmp,
                            float(EC), float(ESC),
                            op0=ALU.subtract, op1=ALU.mult)
                        pair = p2_acc.tile([P, QB], BF16, name="pair",
                                           tag="acc")
                        nc.gpsimd.tensor_add(pair, expt_tiles[kp][:, 0],
                                             expt_tiles[kp][:, 1])
                        pairs.append(pair)
                    while len(pairs) > 1:
                        nxt = []
                        for a, b in zip(pairs[0::2], pairs[1::2]):
                            nacc = p2_acc.tile([P, QB], BF16, name="acc",
                                               tag="acc")
                            nc.vector.tensor_add(nacc, a, b)
                            nxt.append(nacc)
                        pairs = nxt
                    return pairs[0]

                def outp_col(cc):
                    po = ps_po.tile([P, QB], F32, name="po", tag="po")
                    for kp in range(KT // 2):
                        nc.tensor.matmul(
                            po,
                            ctxk_sb[:, 2 * kp:2 * kp + 2,
                                    cc * P:(cc + 1) * P],
                            expt_tiles[kp],
                            start=(kp == 0), stop=(kp == KT // 2 - 1),
                            perf_mode=DR)
                    nc.scalar.activation(
                        outp_sb[:, cc], po, AF.Identity,
                        scale=1.0 / ESC, bias=csum_sb[:, cc:cc + 1])

                def outp_block(acc):
                    outp_col(0)
                    outp_col(1)
                    outp_col(2)
                    psums = ps_fin.tile([P, QB], F32, name="psums", tag="pf")
                    nc.tensor.matmul(psums, om_sb, acc, start=True, stop=True)
                    nc.vector.tensor_copy(sums_sb, psums)
                    outp_col(3)
                    prt = ps_fin.tile([P, 8], F32, name="prt", tag="pf")
                    for qs in range(4):
                        nc.tensor.matmul(
                            prt[:, 2 * qs:2 * qs + 2],
                            sums_sb[:, qs * P:(qs + 1) * P], e0_sb,
                            start=True, stop=True)
                    rtmp = p2_acc.tile([P, 8], F32, name="rtmp", tag="rtmp")
                    nc.vector.tensor_scalar(
                        rtmp, prt, 1.0 / ESC, 2048.0 * EC,
                        op0=ALU.mult, op1=ALU.add)
                    recip = p2_acc.tile([P, 8], F32, name="recip",
                                        tag="recip")
                    nc.vector.reciprocal(recip, rtmp)
                    outp_col(4)
                    outp_col(5)
                    return recip

                def finals_block(qb, recip):
                    for qs in range(4):
                        for oc in range(2):
                            pf = ps_fin.tile([P, 512], F32, name="pf",
                                             tag="pf")
                            for cs in range(CT):
                                nc.tensor.matmul(
                                    pf, outp_sb[:, cs, qs * P:(qs + 1) * P],
                                    wvo_sb[:, cs, oc * 512:(oc + 1) * 512],
                                    start=(cs == 0), stop=(cs == CT - 1))
                            seng = (nc.sync, nc.gpsimd)[(qs * 2 + oc) % 2]
                            rows = slice(qb * QB + qs * P,
                                         qb * QB + (qs + 1) * P)
                            # last group: half-width granules shorten the
                            # serial STT->DMA tail after the last matmul
                            nh = 2 if (qb == NQB - 1 and qs == 3) else 1
                            w = 512 // nh
                            for h in range(nh):
                                cl = oc * 512 + h * w
                                fin = p2_acc.tile([P, w], BF16, name="fin",
                                                  tag="fin")
                                nc.vector.scalar_tensor_tensor(
                                    fin, pf[:, h * w:(h + 1) * w],
                                    recip[:, 2 * qs:2 * qs + 1],
                                    bo_sb[:, cl:cl + w],
                                    op0=ALU.mult, op1=ALU.add)
                                seng.dma_start(out[rows, cl:cl + w], fin)

                # software-pipelined qb loop: finals(qb-1) run between
                # scores(qb) and outp(qb), so the tail of the Exp->center
                # chain (which outp_col(0) would otherwise catch up with)
                # drains under 10.8us of final matmuls.
                prev = None
                for qb in range(NQB):
                    acc = scores_block(qb)
                    if prev is not None:
                        finals_block(*prev)
                    recip = outp_block(acc)
                    prev = (qb, recip)
                finals_block(*prev)
    nc.finalize()
    _NC_CACHE["nc"] = nc
    return nc


def _host_prep(x, context, Wq, bq, Wk, bk, Wv, bv, Wo, bo):
    """Build the 8 per-core input maps (host-side weight folding)."""
    BF = ml_dtypes.bfloat16
    F8np = ml_dtypes.float8_e4m3
    x = np.asarray(x, dtype=np.float32)
    context = np.asarray(context, dtype=np.float32)
    Wq64 = np.asarray(Wq, np.float64)
    Wk64 = np.asarray(Wk, np.float64)
    Wv64 = np.asarray(Wv, np.float64)
    Wo64 = np.asarray(Wo, np.float64)
    M = Wq64.T @ Wk64                                 # [D, C]
    bqk = np.asarray(bq, np.float64) @ Wk64           # [C]
    WVO = (Wo64 @ Wv64).T                             # [C, D]
    bo_eff = np.asarray(bo, np.float64) + Wo64 @ np.asarray(bv, np.float64)

    Mh = np.ascontiguousarray(M.astype(np.float32)).astype(BF)
    wvoh = np.ascontiguousarray(WVO.astype(np.float32)).astype(BF)
    bqkh = np.ascontiguousarray(
        bqk.astype(np.float32).reshape(CT, P).T)      # [p, ct]
    bob = np.ascontiguousarray(
        np.broadcast_to(bo_eff.astype(np.float32)[None, :], (P, D)))
    onesmat = np.ones((P, P), np.float32).astype(BF)
    e0two = np.zeros((P, 2), np.float32)
    e0two[0, :] = 1.0
    e0two = e0two.astype(BF)
    ecbh = np.full((P, 1), -ESC * EC, np.float32)
    shared = dict(Mh=Mh, wvoh=wvoh, bqkh=bqkh, bob=bob,
                  onesmat=onesmat, e0two=e0two, ecbh=ecbh)
    xbf = x.astype(BF)
    in_maps = []
    for b in range(B):
        m = dict(shared)
        m["xT"] = np.ascontiguousarray(xbf[b].T)              # [D, SQ] bf16
        m["ctx8T"] = np.ascontiguousarray(context[b].T).astype(F8np)
        ck8 = np.ascontiguousarray(context[b]).astype(F8np)   # [SKV, C]
        m["ctxk8"] = ck8
        csum = EC * ck8.astype(np.float32).sum(axis=0)        # [C]
        m["csumb"] = np.ascontiguousarray(csum.reshape(CT, P).T)
        in_maps.append(m)
    return in_maps


def kernel(**inputs) -> np.ndarray:
    nc = build()
    in_maps = _host_prep(**inputs)
    res = run_bass_kernel_spmd(nc, in_maps, core_ids=list(range(B)))
    return np.stack(
        [res.results[b]["out"].astype(np.float32) for b in range(B)], axis=0)
